# revision 27
# baseline (speedup 1.0000x reference)
"""Trainium2 Bass kernel for conv-qkv rank-1 attention.

out = gamma * q * sum((k+bk)*(v+bv)) + x, where q,k,v are per-time-slice
3x3 convs (C=64 -> C=64) of x [B=8, C=64, T=16, W=64, H=64].

Sharding: data-parallel over B across 8 cores (1 example/core), conv
weights replicated. No cross-core communication.

Per-core schedule (v2, tap-paired bf16):
Each slice keeps TWO copies of x in one SBUF tile [128, 66, 66]:
even slices [shift | plain], odd slices [plain | shift], where "shift"
is x offset one column so that a single 128-partition moving AP delivers
two different conv taps on the two partition halves. A 3x3 conv then
costs 3 K=128 "pair" matmuls (taps (dy,0)+(dy,1)) plus 3 K=64 "single"
matmuls (taps (dy,2)); singles of even/odd slices sit on disjoint PE
row groups and run concurrently (2-way row tiling), v-chain matmuls of
even/odd slices sit on disjoint column groups (2-way col tiling).
Per block of 512 pixels and slice pair this is 15 PE slots of N=512
vs 20 in the direct scheme.

Biases never enter the PE: bq/bv are folded into the PSUM->SBUF
evacuation on ScalarE (activation bias), and bk's contribution
bk*sum(v+bv) is recovered from the v-evacuation's accum_out.
The final out = q*(gamma*s) + x runs on GpSimd (Pool) so the DVE
queue never stalls the next pair's PSUM turnaround.

All matmul operands are bf16 (hosts casts x with round-to-nearest);
PSUM accumulation stays fp32.
"""

import os

import numpy as np
import ml_dtypes

import concourse.bacc as bacc
import concourse.bass as bass
import concourse.mybir as mybir
import concourse.tile as tile
from concourse import bass_utils

F32 = mybir.dt.float32
BF16 = mybir.dt.bfloat16
ALU = mybir.AluOpType
ACT = mybir.ActivationFunctionType

B, C, T, W, H = 8, 64, 16, 64, 64
WP, HP = W + 2, H + 2          # padded slice dims
NPAIR = int(os.environ.get("BASS_NPAIR", T // 2))  # slice pairs per core
RB = 8                         # W-rows per pixel block
NBLK = W // RB                 # pixel blocks per slice
BN = RB * H                    # moving free dim per matmul (512)
NABUF = 4                      # A-tile buffers per parity (4-deep rotation)
FINCH = 4                      # final-pass chunks per slice


def _bf16(a):
    return np.asarray(a, np.float32).astype(ml_dtypes.bfloat16)


def _pack_weights(wq, wk, wv):
    """Pack stationary operands (bf16).

    Moving-data convention: a pair matmul reads the full 128-partition AP
    at (r0=j*RB+dy, dx=0): on even slices the low half (shift copy)
    delivers tap (dy,1) and the high half (plain) tap (dy,0); odd slices
    are mirrored. Single matmuls read the shift copy at dx=1 -> tap
    (dy,2): even from partitions 0-63, odd from 64-127.
    kq column layout: even [Wk | Wq] (q lands on PSUM 64:128 = the
    x-plain half), odd [Wq | Wk].
    """
    def taps(w):  # [O, I, 1, 3, 3] -> tap(dy,dx) = [I, O]
        return np.ascontiguousarray(w.reshape(C, C, 3, 3).transpose(1, 2, 3, 0),
                                    np.float32)

    q_t, k_t, v_t = taps(wq), taps(wk), taps(wv)

    kq_pair = np.zeros((2, 3, 128, 128), np.float32)
    v_pair = np.zeros((2, 3, 128, 64), np.float32)
    for dy in range(3):
        # even parity
        kq_pair[0, dy, 0:64, 0:64] = k_t[:, dy, 1]
        kq_pair[0, dy, 0:64, 64:128] = q_t[:, dy, 1]
        kq_pair[0, dy, 64:128, 0:64] = k_t[:, dy, 0]
        kq_pair[0, dy, 64:128, 64:128] = q_t[:, dy, 0]
        v_pair[0, dy, 0:64, :] = v_t[:, dy, 1]
        v_pair[0, dy, 64:128, :] = v_t[:, dy, 0]
        # odd parity
        kq_pair[1, dy, 0:64, 0:64] = q_t[:, dy, 0]
        kq_pair[1, dy, 0:64, 64:128] = k_t[:, dy, 0]
        kq_pair[1, dy, 64:128, 0:64] = q_t[:, dy, 1]
        kq_pair[1, dy, 64:128, 64:128] = k_t[:, dy, 1]
        v_pair[1, dy, 0:64, :] = v_t[:, dy, 0]
        v_pair[1, dy, 64:128, :] = v_t[:, dy, 1]

    kq_sing = np.zeros((3, 128, 128), np.float32)
    v_sing = np.zeros((3, 128, 64), np.float32)
    for dy in range(3):
        kq_sing[dy, 0:64, 0:64] = k_t[:, dy, 2]
        kq_sing[dy, 0:64, 64:128] = q_t[:, dy, 2]
        kq_sing[dy, 64:128, 0:64] = q_t[:, dy, 2]
        kq_sing[dy, 64:128, 64:128] = k_t[:, dy, 2]
        v_sing[dy, 0:64, :] = v_t[:, dy, 2]
        v_sing[dy, 64:128, :] = v_t[:, dy, 2]

    return _bf16(kq_pair), _bf16(kq_sing), _bf16(v_pair), _bf16(v_sing)


def _emit(nc, tc, x_d, xp_d, wkqp_d, wkqs_d, wvp_d, wvs_d, bias_d, out_d,
          ctx):
    const = ctx.enter_context(tc.tile_pool(name="const", bufs=1))
    state = ctx.enter_context(tc.tile_pool(name="state", bufs=1))
    psum = ctx.enter_context(
        tc.tile_pool(name="psum", bufs=3, space=bass.MemorySpace.PSUM))
    psumv = ctx.enter_context(
        tc.tile_pool(name="psumv", bufs=2, space=bass.MemorySpace.PSUM))
    vpool = ctx.enter_context(tc.tile_pool(name="vpool", bufs=2))

    wkqp_t = const.tile([128, 2, 3, 128], BF16, tag="wkqp")
    wkqs_t = const.tile([128, 3, 128], BF16, tag="wkqs")
    wvp_t = const.tile([128, 2, 3, 64], BF16, tag="wvp")
    wvs_t = const.tile([128, 3, 64], BF16, tag="wvs")
    bias_t = const.tile([128, 4], F32, tag="bias")  # bq, bv, bk*gam, gam

    nc.scalar.dma_start(bias_t[:], bias_d[:])
    nc.scalar.dma_start(wkqp_t[:], wkqp_d[:])
    nc.scalar.dma_start(wkqs_t[:], wkqs_d[:])
    nc.scalar.dma_start(wvp_t[:], wvp_d[:])
    nc.scalar.dma_start(wvs_t[:], wvs_d[:])

    # A tiles: [shift | plain] for even slices, [plain | shift] for odd.
    ae = [state.tile([128, WP, HP], BF16, tag=f"ae{i}", name=f"ae{i}")
          for i in range(NABUF)]
    ao = [state.tile([128, WP, HP], BF16, tag=f"ao{i}", name=f"ao{i}")
          for i in range(NABUF)]
    qs = [state.tile([128, W * H], BF16, tag=f"qs{i}", name=f"qs{i}")
          for i in range(2)]
    ot = [state.tile([128, W * H], BF16, tag=f"ot{i}", name=f"ot{i}")
          for i in range(2)]
    scr = state.tile([128, BN], F32, tag="scr")
    sparts = [state.tile([128, NBLK], F32, tag=f"sp{i}", name=f"sp{i}")
              for i in range(2)]
    vsum = [state.tile([128, NBLK], F32, tag=f"vs{i}", name=f"vs{i}")
            for i in range(2)]
    t1 = [state.tile([128, 1], F32, tag=f"t1{i}", name=f"t1{i}")
          for i in range(2)]
    vs1 = [state.tile([128, 1], F32, tag=f"vs1{i}", name=f"vs1{i}")
           for i in range(2)]
    sg = [state.tile([128, 1], F32, tag=f"sg{i}", name=f"sg{i}")
          for i in range(2)]
    sf = [state.tile([128, 1], F32, tag=f"sf{i}", name=f"sf{i}")
          for i in range(2)]

    # plain-x tile for the full-width final pass: odd slice on partitions
    # 0-63, even slice on 64-127 (matches q/s/out halves)
    xpl = [state.tile([128, W * H], BF16, tag=f"xpl{i}", name=f"xpl{i}")
           for i in range(NABUF)]

    def load_pair(p, xpl_eng=None):
        # host pre-padded slices: one contiguous descriptor per partition
        nc.sync.dma_start(ae[p % NABUF][:], xp_d[2 * p])
        nc.gpsimd.dma_start(ao[p % NABUF][:], xp_d[2 * p + 1])
        xq = xpl_eng or nc.sync
        xq.dma_start(xpl[p % NABUF][0:64, :], x_d[:, 2 * p + 1])
        xq.dma_start(xpl[p % NABUF][64:128, :], x_d[:, 2 * p])

    # startup: ae loads lead on sync, xpl trails on gpsimd so the first
    # blocks' matmuls start as soon as ae0/ao0 land
    load_pair(0, xpl_eng=nc.gpsimd)
    if NPAIR > 1:
        load_pair(1, xpl_eng=nc.gpsimd)

    kvo_mode = os.environ.get("BASS_KVO", "stt")
    fin_eng = nc.vector  # Pool lacks TensorScalarPtr; bf16 gives 2x DVE rate

    def emit_final(p, nch=FINCH):
        # out = q*(gamma*s) + x, full 128-partition DVE ops (both slices
        # at once; xpl interleaves odd/even plain x), store per chunk
        pb = p % 2
        cw = W // nch
        cn = cw * H
        for c_ in range(nch):
            fin_eng.scalar_tensor_tensor(
                out=ot[pb][:, c_ * cn:(c_ + 1) * cn],
                in0=qs[pb][:, c_ * cn:(c_ + 1) * cn],
                scalar=sf[pb][:, 0:1],
                in1=xpl[p % NABUF][:, c_ * cn:(c_ + 1) * cn],
                op0=ALU.mult, op1=ALU.add)
            nc.gpsimd.dma_start(out_d[:, 2 * p, c_ * cw:(c_ + 1) * cw, :],
                                ot[pb][64:128, c_ * cn:(c_ + 1) * cn])
            nc.gpsimd.dma_start(out_d[:, 2 * p + 1, c_ * cw:(c_ + 1) * cw, :],
                                ot[pb][0:64, c_ * cn:(c_ + 1) * cn])

    for p in range(NPAIR):
        pb = p % 2
        ae_, ao_ = ae[p % NABUF], ao[p % NABUF]
        qs_, ot_ = qs[pb], ot[pb]

        for j in range(NBLK):
            kqE = psum.tile([128, BN], F32, tag="kqE", name="kqE")
            kqO = psum.tile([128, BN], F32, tag="kqO", name="kqO")
            V = psumv.tile([128, BN], F32, tag="V", name="V")

            def mov(tl, dy, dx, lo=None):
                r0 = j * RB + dy
                if lo is None:
                    return tl[:, r0:r0 + RB, dx:dx + H]
                if lo:
                    return tl[0:64, r0:r0 + RB, dx:dx + H]
                return tl[64:128, r0:r0 + RB, dx:dx + H]

            # kq pair taps: K=128, full array, serial
            for dy in range(3):
                nc.tensor.matmul(kqE[:, :], wkqp_t[:, 0, dy, :],
                                 mov(ae_, dy, 0), start=(dy == 0), stop=False)
                nc.tensor.matmul(kqO[:, :], wkqp_t[:, 1, dy, :],
                                 mov(ao_, dy, 0), start=(dy == 0), stop=False)
            # kq single taps: K=64, even rows 0-63 / odd rows 64-127,
            # 2-way row-tiled concurrent
            for dy in range(3):
                nc.tensor.matmul(kqE[:, :], wkqs_t[0:64, dy, :],
                                 mov(ae_, dy, 1, lo=True),
                                 start=False, stop=(dy == 2))
                nc.tensor.matmul(kqO[:, :], wkqs_t[64:128, dy, :],
                                 mov(ao_, dy, 1, lo=False),
                                 start=False, stop=(dy == 2))
            # v pair taps: K=128 M=64, even cols 0-63 / odd cols 64-127,
            # 2-way col-tiled concurrent
            for dy in range(3):
                nc.tensor.matmul(V[0:64, :], wvp_t[:, 0, dy, :],
                                 mov(ae_, dy, 0), start=(dy == 0), stop=False,
                                 skip_group_check=True)
                nc.tensor.matmul(V[64:128, :], wvp_t[:, 1, dy, :],
                                 mov(ao_, dy, 0), start=(dy == 0), stop=False,
                                 skip_group_check=True)
            # v single taps: K=64 M=64, quadrants (0,0) and (64,64)
            for dy in range(3):
                nc.tensor.matmul(V[0:64, :], wvs_t[0:64, dy, :],
                                 mov(ae_, dy, 1, lo=True),
                                 start=False, stop=(dy == 2),
                                 skip_group_check=True)
                nc.tensor.matmul(V[64:128, :], wvs_t[64:128, dy, :],
                                 mov(ao_, dy, 1, lo=False),
                                 start=False, stop=(dy == 2),
                                 skip_group_check=True)

            # PSUM evacuation on ScalarE with bias folding
            nc.scalar.activation(qs_[64:128, j * BN:(j + 1) * BN],
                                 kqE[64:128, :], ACT.Identity,
                                 bias=bias_t[64:128, 0:1])
            nc.scalar.activation(qs_[0:64, j * BN:(j + 1) * BN],
                                 kqO[0:64, :], ACT.Identity,
                                 bias=bias_t[0:64, 0:1])
            vsb = vpool.tile([128, BN], F32, tag="vsb", name="vsb")
            nc.scalar.activation(vsb[:, :], V[:, :], ACT.Identity,
                                 bias=bias_t[:, 1:2],
                                 accum_out=vsum[pb][:, j:j + 1])

            # fused (gamma*k)*v' multiply + pixel-sum on DVE
            nc.vector.scalar_tensor_tensor(
                out=scr[0:64, :], in0=kqE[0:64, :], scalar=bias_t[0:64, 3:4],
                in1=vsb[0:64, :], op0=ALU.mult, op1=ALU.mult,
                accum_out=sparts[pb][0:64, j:j + 1])
            if kvo_mode == "stt":
                nc.vector.scalar_tensor_tensor(
                    out=scr[64:128, :], in0=kqO[64:128, :],
                    scalar=bias_t[64:128, 3:4],
                    in1=vsb[64:128, :], op0=ALU.mult, op1=ALU.mult,
                    accum_out=sparts[pb][64:128, j:j + 1])
            else:
                nc.vector.scalar_tensor_tensor(
                    out=scr[64:128, :], in0=kqO[64:128, :],
                    scalar=bias_t[64:128, 3:4],
                    in1=vsb[64:128, :], op0=ALU.mult, op1=ALU.mult)
                nc.vector.reduce_sum(sparts[pb][64:128, j:j + 1],
                                     scr[64:128, :],
                                     axis=mybir.AxisListType.X)

            # previous pair's final pass, deferred so the DVE queue never
            # holds this pair's psum turnaround behind it; prefetch loads
            # likewise emitted mid-pair
            if j == 1 and p > 0:
                emit_final(p - 1)
            if j == 3 and p + 2 < NPAIR:
                load_pair(p + 2)

        # s = gamma*sum(k*v') + gamma*bk*sum(v'), then swap halves so the
        # even-slice s reaches partitions 64-127 (q/x-plain live there)
        nc.vector.reduce_sum(t1[pb][:, :], sparts[pb][:, :],
                             axis=mybir.AxisListType.X)
        nc.vector.reduce_sum(vs1[pb][:, :], vsum[pb][:, :],
                             axis=mybir.AxisListType.X)
        nc.vector.scalar_tensor_tensor(
            out=sg[pb][:, :], in0=vs1[pb][:, :], scalar=bias_t[:, 2:3],
            in1=t1[pb][:, :], op0=ALU.mult, op1=ALU.add)
        nc.scalar.dma_start(sf[pb][64:128, :], sg[pb][0:64, :])
        nc.scalar.dma_start(sf[pb][0:64, :], sg[pb][64:128, :])

    emit_final(NPAIR - 1, nch=8)   # small chunks: fast epilogue drain


_CACHE = {}


def _build():
    if "nc" in _CACHE:
        return _CACHE["nc"]
    nc = bacc.Bacc("TRN2", target_bir_lowering=False, debug=False,
                   enable_asserts=False, num_devices=8)
    x_d = nc.dram_tensor("x", (C, T, W, H), BF16, kind="ExternalInput").ap()
    xp_d = nc.dram_tensor("xp", (T, 128, WP, HP), BF16,
                          kind="ExternalInput").ap()
    wkqp_d = nc.dram_tensor("wkqp", (128, 2, 3, 128), BF16,
                            kind="ExternalInput").ap()
    wkqs_d = nc.dram_tensor("wkqs", (128, 3, 128), BF16,
                            kind="ExternalInput").ap()
    wvp_d = nc.dram_tensor("wvp", (128, 2, 3, 64), BF16,
                           kind="ExternalInput").ap()
    wvs_d = nc.dram_tensor("wvs", (128, 3, 64), BF16,
                           kind="ExternalInput").ap()
    bias_d = nc.dram_tensor("bias", (128, 4), F32, kind="ExternalInput").ap()
    out_d = nc.dram_tensor("out", (C, T, W, H), BF16,
                           kind="ExternalOutput").ap()
    from contextlib import ExitStack
    with tile.TileContext(nc) as tc, ExitStack() as ctx:
        _emit(nc, tc, x_d, xp_d, wkqp_d, wkqs_d, wvp_d, wvs_d, bias_d, out_d,
              ctx)
    nc.compile()
    _CACHE["nc"] = nc
    return nc


def run_spmd(x, wq, wk, wv, bq, bk, bv, gamma, trace=False, **kw):
    nc = _build()
    wkqp, wkqs, wvp, wvs = _pack_weights(
        np.asarray(wq, np.float32), np.asarray(wk, np.float32),
        np.asarray(wv, np.float32))
    # stationary tiles are [128(K), ...free...]: transpose packed
    # [..., K, M] so K is the partition dim
    wkqp = np.ascontiguousarray(wkqp.transpose(2, 0, 1, 3))   # [128,2,3,128]
    wkqs = np.ascontiguousarray(wkqs.transpose(1, 0, 2))      # [128,3,128]
    wvp = np.ascontiguousarray(wvp.transpose(2, 0, 1, 3))     # [128,2,3,64]
    wvs = np.ascontiguousarray(wvs.transpose(1, 0, 2))        # [128,3,64]

    gam = np.float32(np.asarray(gamma).reshape(-1)[0])
    bias = np.zeros((128, 4), np.float32)
    bias[0:64, 0] = bias[64:128, 0] = np.asarray(bq, np.float32)
    bias[0:64, 1] = bias[64:128, 1] = np.asarray(bv, np.float32)
    bias[0:64, 2] = bias[64:128, 2] = np.asarray(bk, np.float32) * gam
    bias[:, 3] = gam

    xb = _bf16(x)
    # host pre-padded per-slice layout [T, 128, WP, HP]: even slices
    # [shift | plain], odd slices [plain | shift] on the partition halves
    zpad = np.zeros((B, T, C, WP, HP), ml_dtypes.bfloat16)
    zsh = np.zeros((B, T, C, WP, HP), ml_dtypes.bfloat16)
    xt = xb.transpose(0, 2, 1, 3, 4)            # [B, T, C, W, H]
    zpad[:, :, :, 1:1 + W, 1:1 + H] = xt
    zsh[:, :, :, 1:1 + W, 0:H] = xt
    xp = np.empty((B, T, 128, WP, HP), ml_dtypes.bfloat16)
    xp[:, 0::2, 0:64] = zsh[:, 0::2]
    xp[:, 0::2, 64:128] = zpad[:, 0::2]
    xp[:, 1::2, 0:64] = zpad[:, 1::2]
    xp[:, 1::2, 64:128] = zsh[:, 1::2]
    in_maps = [
        {"x": np.ascontiguousarray(xb[b]), "xp": np.ascontiguousarray(xp[b]),
         "wkqp": wkqp, "wkqs": wkqs,
         "wvp": wvp, "wvs": wvs, "bias": bias}
        for b in range(B)
    ]
    res = bass_utils.run_bass_kernel_spmd(
        nc, in_maps, core_ids=list(range(B)), trace=trace, **kw)
    out = np.stack([np.asarray(res.results[b]["out"]).astype(np.float32)
                    for b in range(B)], axis=0)
    return out, res


def kernel(x, wq, wk, wv, bq, bk, bv, gamma):
    out, _ = run_spmd(x, wq, wk, wv, bq, bk, bv, gamma)
    return out


# revision 31
# speedup vs baseline: 1.0912x; 1.0912x over previous
"""Trainium2 Bass kernel for conv-qkv rank-1 attention.

out = gamma * q * sum((k+bk)*(v+bv)) + x, where q,k,v are per-time-slice
3x3 convs (C=64 -> C=64) of x [B=8, C=64, T=16, W=64, H=64].

Sharding: data-parallel over B across 8 cores (1 example/core), conv
weights replicated. No cross-core communication.

Per-core schedule (v2, tap-paired bf16):
Each slice keeps TWO copies of x in one SBUF tile [128, 66, 66]:
even slices [shift | plain], odd slices [plain | shift], where "shift"
is x offset one column so that a single 128-partition moving AP delivers
two different conv taps on the two partition halves. A 3x3 conv then
costs 3 K=128 "pair" matmuls (taps (dy,0)+(dy,1)) plus 3 K=64 "single"
matmuls (taps (dy,2)); singles of even/odd slices sit on disjoint PE
row groups and run concurrently (2-way row tiling), v-chain matmuls of
even/odd slices sit on disjoint column groups (2-way col tiling).
Per block of 512 pixels and slice pair this is 15 PE slots of N=512
vs 20 in the direct scheme.

Biases never enter the PE: bq/bv are folded into the PSUM->SBUF
evacuation on ScalarE (activation bias), and bk's contribution
bk*sum(v+bv) is recovered from the v-evacuation's accum_out.
The final out = q*(gamma*s) + x runs on GpSimd (Pool) so the DVE
queue never stalls the next pair's PSUM turnaround.

All matmul operands are bf16 (hosts casts x with round-to-nearest);
PSUM accumulation stays fp32.
"""

import os

import numpy as np
import ml_dtypes

import concourse.bacc as bacc
import concourse.bass as bass
import concourse.mybir as mybir
import concourse.tile as tile
from concourse import bass_utils

F32 = mybir.dt.float32
BF16 = mybir.dt.bfloat16
ALU = mybir.AluOpType
ACT = mybir.ActivationFunctionType

B, C, T, W, H = 8, 64, 16, 64, 64
WP, HP = W + 2, H + 2          # padded slice dims
NPAIR = int(os.environ.get("BASS_NPAIR", T // 2))  # slice pairs per core
RB = 8                         # W-rows per pixel block
NBLK = W // RB                 # pixel blocks per slice
BN = RB * H                    # moving free dim per matmul (512)
NABUF = 4                      # A-tile buffers per parity (4-deep rotation)
FINCH = 4                      # final-pass chunks per slice


def _bf16(a):
    return np.asarray(a, np.float32).astype(ml_dtypes.bfloat16)


def _pack_weights(wq, wk, wv):
    """Pack stationary operands (bf16).

    Moving-data convention: a pair matmul reads the full 128-partition AP
    at (r0=j*RB+dy, dx=0): on even slices the low half (shift copy)
    delivers tap (dy,1) and the high half (plain) tap (dy,0); odd slices
    are mirrored. Single matmuls read the shift copy at dx=1 -> tap
    (dy,2): even from partitions 0-63, odd from 64-127.
    kq column layout: even [Wk | Wq] (q lands on PSUM 64:128 = the
    x-plain half), odd [Wq | Wk].
    """
    def taps(w):  # [O, I, 1, 3, 3] -> tap(dy,dx) = [I, O]
        return np.ascontiguousarray(w.reshape(C, C, 3, 3).transpose(1, 2, 3, 0),
                                    np.float32)

    q_t, k_t, v_t = taps(wq), taps(wk), taps(wv)

    kq_pair = np.zeros((2, 3, 128, 128), np.float32)
    v_pair = np.zeros((2, 3, 128, 64), np.float32)
    for dy in range(3):
        # even parity
        kq_pair[0, dy, 0:64, 0:64] = k_t[:, dy, 1]
        kq_pair[0, dy, 0:64, 64:128] = q_t[:, dy, 1]
        kq_pair[0, dy, 64:128, 0:64] = k_t[:, dy, 0]
        kq_pair[0, dy, 64:128, 64:128] = q_t[:, dy, 0]
        v_pair[0, dy, 0:64, :] = v_t[:, dy, 1]
        v_pair[0, dy, 64:128, :] = v_t[:, dy, 0]
        # odd parity
        kq_pair[1, dy, 0:64, 0:64] = q_t[:, dy, 0]
        kq_pair[1, dy, 0:64, 64:128] = k_t[:, dy, 0]
        kq_pair[1, dy, 64:128, 0:64] = q_t[:, dy, 1]
        kq_pair[1, dy, 64:128, 64:128] = k_t[:, dy, 1]
        v_pair[1, dy, 0:64, :] = v_t[:, dy, 0]
        v_pair[1, dy, 64:128, :] = v_t[:, dy, 1]

    kq_sing = np.zeros((3, 128, 128), np.float32)
    v_sing = np.zeros((3, 128, 64), np.float32)
    for dy in range(3):
        kq_sing[dy, 0:64, 0:64] = k_t[:, dy, 2]
        kq_sing[dy, 0:64, 64:128] = q_t[:, dy, 2]
        kq_sing[dy, 64:128, 0:64] = q_t[:, dy, 2]
        kq_sing[dy, 64:128, 64:128] = k_t[:, dy, 2]
        v_sing[dy, 0:64, :] = v_t[:, dy, 2]
        v_sing[dy, 64:128, :] = v_t[:, dy, 2]

    return _bf16(kq_pair), _bf16(kq_sing), _bf16(v_pair), _bf16(v_sing)


def _emit(nc, tc, x_d, xp_d, wkqp_d, wkqs_d, wvp_d, wvs_d, bias_d, out_d,
          ctx):
    const = ctx.enter_context(tc.tile_pool(name="const", bufs=1))
    state = ctx.enter_context(tc.tile_pool(name="state", bufs=1))
    psum = ctx.enter_context(
        tc.tile_pool(name="psum", bufs=3, space=bass.MemorySpace.PSUM))
    psumv = ctx.enter_context(
        tc.tile_pool(name="psumv", bufs=2, space=bass.MemorySpace.PSUM))
    vpool = ctx.enter_context(tc.tile_pool(name="vpool", bufs=2))

    wkqp_t = const.tile([128, 2, 3, 128], BF16, tag="wkqp")
    wkqs_t = const.tile([128, 3, 128], BF16, tag="wkqs")
    wvp_t = const.tile([128, 2, 3, 64], BF16, tag="wvp")
    wvs_t = const.tile([128, 3, 64], BF16, tag="wvs")
    bias_t = const.tile([128, 4], F32, tag="bias")  # bq, bv, bk*gam, gam

    nc.sync.dma_start(wkqp_t[:], wkqp_d[:])
    nc.sync.dma_start(wkqs_t[:], wkqs_d[:])
    nc.sync.dma_start(wvp_t[:], wvp_d[:])
    nc.sync.dma_start(wvs_t[:], wvs_d[:])
    nc.sync.dma_start(bias_t[:], bias_d[:])

    # A tiles: [shift | plain] for even slices, [plain | shift] for odd.
    ae = [state.tile([128, WP, HP], BF16, tag=f"ae{i}", name=f"ae{i}")
          for i in range(NABUF)]
    ao = [state.tile([128, WP, HP], BF16, tag=f"ao{i}", name=f"ao{i}")
          for i in range(NABUF)]
    qs = [state.tile([128, W * H], BF16, tag=f"qs{i}", name=f"qs{i}")
          for i in range(2)]
    ot = [state.tile([128, W * H], BF16, tag=f"ot{i}", name=f"ot{i}")
          for i in range(2)]
    scr = state.tile([128, BN], F32, tag="scr")
    sparts = [state.tile([128, NBLK], F32, tag=f"sp{i}", name=f"sp{i}")
              for i in range(2)]
    vsum = [state.tile([128, NBLK], F32, tag=f"vs{i}", name=f"vs{i}")
            for i in range(2)]
    t1 = [state.tile([128, 1], F32, tag=f"t1{i}", name=f"t1{i}")
          for i in range(2)]
    vs1 = [state.tile([128, 1], F32, tag=f"vs1{i}", name=f"vs1{i}")
           for i in range(2)]
    sg = [state.tile([128, 1], F32, tag=f"sg{i}", name=f"sg{i}")
          for i in range(2)]
    sf = [state.tile([128, 1], F32, tag=f"sf{i}", name=f"sf{i}")
          for i in range(2)]

    # plain-x tile for the full-width final pass: odd slice on partitions
    # 0-63, even slice on 64-127 (matches q/s/out halves)
    xpl = [state.tile([128, W * H], BF16, tag=f"xpl{i}", name=f"xpl{i}")
           for i in range(NABUF)]

    def load_pair(p, xpl_eng=None):
        # host pre-padded slices: one contiguous descriptor per partition
        nc.sync.dma_start(ae[p % NABUF][:], xp_d[2 * p])
        nc.gpsimd.dma_start(ao[p % NABUF][:], xp_d[2 * p + 1])
        xq = xpl_eng or nc.sync
        xq.dma_start(xpl[p % NABUF][0:64, :], x_d[:, 2 * p + 1])
        xq.dma_start(xpl[p % NABUF][64:128, :], x_d[:, 2 * p])

    load_pair(0)
    if NPAIR > 1:
        load_pair(1)

    kvo_mode = os.environ.get("BASS_KVO", "stt")
    fin_eng = nc.vector  # Pool lacks TensorScalarPtr; bf16 gives 2x DVE rate

    def emit_final(p, nch=FINCH):
        # out = q*(gamma*s) + x, full 128-partition DVE ops (both slices
        # at once; xpl interleaves odd/even plain x), store per chunk
        pb = p % 2
        cw = W // nch
        cn = cw * H
        for c_ in range(nch):
            fin_eng.scalar_tensor_tensor(
                out=ot[pb][:, c_ * cn:(c_ + 1) * cn],
                in0=qs[pb][:, c_ * cn:(c_ + 1) * cn],
                scalar=sf[pb][:, 0:1],
                in1=xpl[p % NABUF][:, c_ * cn:(c_ + 1) * cn],
                op0=ALU.mult, op1=ALU.add)
            nc.gpsimd.dma_start(out_d[:, 2 * p, c_ * cw:(c_ + 1) * cw, :],
                                ot[pb][64:128, c_ * cn:(c_ + 1) * cn])
            nc.gpsimd.dma_start(out_d[:, 2 * p + 1, c_ * cw:(c_ + 1) * cw, :],
                                ot[pb][0:64, c_ * cn:(c_ + 1) * cn])

    for p in range(NPAIR):
        pb = p % 2
        ae_, ao_ = ae[p % NABUF], ao[p % NABUF]
        qs_, ot_ = qs[pb], ot[pb]

        for j in range(NBLK):
            kqE = psum.tile([128, BN], F32, tag="kqE", name="kqE")
            kqO = psum.tile([128, BN], F32, tag="kqO", name="kqO")
            V = psumv.tile([128, BN], F32, tag="V", name="V")

            def mov(tl, dy, dx, lo=None):
                r0 = j * RB + dy
                if lo is None:
                    return tl[:, r0:r0 + RB, dx:dx + H]
                if lo:
                    return tl[0:64, r0:r0 + RB, dx:dx + H]
                return tl[64:128, r0:r0 + RB, dx:dx + H]

            # kq pair taps: K=128, full array, serial
            for dy in range(3):
                nc.tensor.matmul(kqE[:, :], wkqp_t[:, 0, dy, :],
                                 mov(ae_, dy, 0), start=(dy == 0), stop=False)
                nc.tensor.matmul(kqO[:, :], wkqp_t[:, 1, dy, :],
                                 mov(ao_, dy, 0), start=(dy == 0), stop=False)
            # kq single taps: K=64, even rows 0-63 / odd rows 64-127,
            # 2-way row-tiled concurrent
            for dy in range(3):
                nc.tensor.matmul(kqE[:, :], wkqs_t[0:64, dy, :],
                                 mov(ae_, dy, 1, lo=True),
                                 start=False, stop=(dy == 2))
                nc.tensor.matmul(kqO[:, :], wkqs_t[64:128, dy, :],
                                 mov(ao_, dy, 1, lo=False),
                                 start=False, stop=(dy == 2))
            # v pair taps: K=128 M=64, even cols 0-63 / odd cols 64-127,
            # 2-way col-tiled concurrent
            for dy in range(3):
                nc.tensor.matmul(V[0:64, :], wvp_t[:, 0, dy, :],
                                 mov(ae_, dy, 0), start=(dy == 0), stop=False,
                                 skip_group_check=True)
                nc.tensor.matmul(V[64:128, :], wvp_t[:, 1, dy, :],
                                 mov(ao_, dy, 0), start=(dy == 0), stop=False,
                                 skip_group_check=True)
            # v single taps: K=64 M=64, quadrants (0,0) and (64,64)
            for dy in range(3):
                nc.tensor.matmul(V[0:64, :], wvs_t[0:64, dy, :],
                                 mov(ae_, dy, 1, lo=True),
                                 start=False, stop=(dy == 2),
                                 skip_group_check=True)
                nc.tensor.matmul(V[64:128, :], wvs_t[64:128, dy, :],
                                 mov(ao_, dy, 1, lo=False),
                                 start=False, stop=(dy == 2),
                                 skip_group_check=True)

            # PSUM evacuation on ScalarE with bias folding; v first — the
            # kv STTs (critical path to freeing kq psum) consume it
            vsb = vpool.tile([128, BN], F32, tag="vsb", name="vsb")
            nc.scalar.activation(vsb[:, :], V[:, :], ACT.Identity,
                                 bias=bias_t[:, 1:2],
                                 accum_out=vsum[pb][:, j:j + 1])
            nc.scalar.activation(qs_[64:128, j * BN:(j + 1) * BN],
                                 kqE[64:128, :], ACT.Identity,
                                 bias=bias_t[64:128, 0:1])
            nc.scalar.activation(qs_[0:64, j * BN:(j + 1) * BN],
                                 kqO[0:64, :], ACT.Identity,
                                 bias=bias_t[0:64, 0:1])

            # fused (gamma*k)*v' multiply + pixel-sum on DVE
            nc.vector.scalar_tensor_tensor(
                out=scr[0:64, :], in0=kqE[0:64, :], scalar=bias_t[0:64, 3:4],
                in1=vsb[0:64, :], op0=ALU.mult, op1=ALU.mult,
                accum_out=sparts[pb][0:64, j:j + 1])
            if kvo_mode == "stt":
                nc.vector.scalar_tensor_tensor(
                    out=scr[64:128, :], in0=kqO[64:128, :],
                    scalar=bias_t[64:128, 3:4],
                    in1=vsb[64:128, :], op0=ALU.mult, op1=ALU.mult,
                    accum_out=sparts[pb][64:128, j:j + 1])
            else:
                nc.vector.scalar_tensor_tensor(
                    out=scr[64:128, :], in0=kqO[64:128, :],
                    scalar=bias_t[64:128, 3:4],
                    in1=vsb[64:128, :], op0=ALU.mult, op1=ALU.mult)
                nc.vector.reduce_sum(sparts[pb][64:128, j:j + 1],
                                     scr[64:128, :],
                                     axis=mybir.AxisListType.X)

            # previous pair's final pass, deferred so the DVE queue never
            # holds this pair's psum turnaround behind it; prefetch loads
            # likewise emitted mid-pair
            if j == 1 and p > 0:
                emit_final(p - 1)
            if j == 3 and p + 2 < NPAIR:
                load_pair(p + 2)

        # s = gamma*sum(k*v') + gamma*bk*sum(v'), then swap halves so the
        # even-slice s reaches partitions 64-127 (q/x-plain live there)
        nc.vector.reduce_sum(t1[pb][:, :], sparts[pb][:, :],
                             axis=mybir.AxisListType.X)
        nc.vector.reduce_sum(vs1[pb][:, :], vsum[pb][:, :],
                             axis=mybir.AxisListType.X)
        nc.vector.scalar_tensor_tensor(
            out=sg[pb][:, :], in0=vs1[pb][:, :], scalar=bias_t[:, 2:3],
            in1=t1[pb][:, :], op0=ALU.mult, op1=ALU.add)
        nc.scalar.dma_start(sf[pb][64:128, :], sg[pb][0:64, :])
        nc.scalar.dma_start(sf[pb][0:64, :], sg[pb][64:128, :])

    emit_final(NPAIR - 1)


_CACHE = {}


def _build():
    if "nc" in _CACHE:
        return _CACHE["nc"]
    nc = bacc.Bacc("TRN2", target_bir_lowering=False, debug=False,
                   enable_asserts=False, num_devices=8)
    x_d = nc.dram_tensor("x", (C, T, W, H), BF16, kind="ExternalInput").ap()
    xp_d = nc.dram_tensor("xp", (T, 128, WP, HP), BF16,
                          kind="ExternalInput").ap()
    wkqp_d = nc.dram_tensor("wkqp", (128, 2, 3, 128), BF16,
                            kind="ExternalInput").ap()
    wkqs_d = nc.dram_tensor("wkqs", (128, 3, 128), BF16,
                            kind="ExternalInput").ap()
    wvp_d = nc.dram_tensor("wvp", (128, 2, 3, 64), BF16,
                           kind="ExternalInput").ap()
    wvs_d = nc.dram_tensor("wvs", (128, 3, 64), BF16,
                           kind="ExternalInput").ap()
    bias_d = nc.dram_tensor("bias", (128, 4), F32, kind="ExternalInput").ap()
    out_d = nc.dram_tensor("out", (C, T, W, H), BF16,
                           kind="ExternalOutput").ap()
    from contextlib import ExitStack
    with tile.TileContext(nc) as tc, ExitStack() as ctx:
        _emit(nc, tc, x_d, xp_d, wkqp_d, wkqs_d, wvp_d, wvs_d, bias_d, out_d,
              ctx)
    nc.compile()
    _CACHE["nc"] = nc
    return nc


def run_spmd(x, wq, wk, wv, bq, bk, bv, gamma, trace=False, **kw):
    nc = _build()
    wkqp, wkqs, wvp, wvs = _pack_weights(
        np.asarray(wq, np.float32), np.asarray(wk, np.float32),
        np.asarray(wv, np.float32))
    # stationary tiles are [128(K), ...free...]: transpose packed
    # [..., K, M] so K is the partition dim
    wkqp = np.ascontiguousarray(wkqp.transpose(2, 0, 1, 3))   # [128,2,3,128]
    wkqs = np.ascontiguousarray(wkqs.transpose(1, 0, 2))      # [128,3,128]
    wvp = np.ascontiguousarray(wvp.transpose(2, 0, 1, 3))     # [128,2,3,64]
    wvs = np.ascontiguousarray(wvs.transpose(1, 0, 2))        # [128,3,64]

    gam = np.float32(np.asarray(gamma).reshape(-1)[0])
    bias = np.zeros((128, 4), np.float32)
    bias[0:64, 0] = bias[64:128, 0] = np.asarray(bq, np.float32)
    bias[0:64, 1] = bias[64:128, 1] = np.asarray(bv, np.float32)
    bias[0:64, 2] = bias[64:128, 2] = np.asarray(bk, np.float32) * gam
    bias[:, 3] = gam

    xb = _bf16(x)
    # host pre-padded per-slice layout [T, 128, WP, HP]: even slices
    # [shift | plain], odd slices [plain | shift] on the partition halves
    zpad = np.zeros((B, T, C, WP, HP), ml_dtypes.bfloat16)
    zsh = np.zeros((B, T, C, WP, HP), ml_dtypes.bfloat16)
    xt = xb.transpose(0, 2, 1, 3, 4)            # [B, T, C, W, H]
    zpad[:, :, :, 1:1 + W, 1:1 + H] = xt
    zsh[:, :, :, 1:1 + W, 0:H] = xt
    xp = np.empty((B, T, 128, WP, HP), ml_dtypes.bfloat16)
    xp[:, 0::2, 0:64] = zsh[:, 0::2]
    xp[:, 0::2, 64:128] = zpad[:, 0::2]
    xp[:, 1::2, 0:64] = zpad[:, 1::2]
    xp[:, 1::2, 64:128] = zsh[:, 1::2]
    in_maps = [
        {"x": np.ascontiguousarray(xb[b]), "xp": np.ascontiguousarray(xp[b]),
         "wkqp": wkqp, "wkqs": wkqs,
         "wvp": wvp, "wvs": wvs, "bias": bias}
        for b in range(B)
    ]
    res = bass_utils.run_bass_kernel_spmd(
        nc, in_maps, core_ids=list(range(B)), trace=trace, **kw)
    out = np.stack([np.asarray(res.results[b]["out"]).astype(np.float32)
                    for b in range(B)], axis=0)
    return out, res


def kernel(x, wq, wk, wv, bq, bk, bv, gamma):
    out, _ = run_spmd(x, wq, wk, wv, bq, bk, bv, gamma)
    return out


# revision 34
# speedup vs baseline: 1.0974x; 1.0057x over previous
"""Trainium2 Bass kernel for conv-qkv rank-1 attention.

out = gamma * q * sum((k+bk)*(v+bv)) + x, where q,k,v are per-time-slice
3x3 convs (C=64 -> C=64) of x [B=8, C=64, T=16, W=64, H=64].

Sharding: data-parallel over B across 8 cores (1 example/core), conv
weights replicated. No cross-core communication.

Per-core schedule (v2, tap-paired bf16):
Each slice keeps TWO copies of x in one SBUF tile [128, 66, 66]:
even slices [shift | plain], odd slices [plain | shift], where "shift"
is x offset one column so that a single 128-partition moving AP delivers
two different conv taps on the two partition halves. A 3x3 conv then
costs 3 K=128 "pair" matmuls (taps (dy,0)+(dy,1)) plus 3 K=64 "single"
matmuls (taps (dy,2)); singles of even/odd slices sit on disjoint PE
row groups and run concurrently (2-way row tiling), v-chain matmuls of
even/odd slices sit on disjoint column groups (2-way col tiling).
Per block of 512 pixels and slice pair this is 15 PE slots of N=512
vs 20 in the direct scheme.

Biases never enter the PE: bq/bv are folded into the PSUM->SBUF
evacuation on ScalarE (activation bias), and bk's contribution
bk*sum(v+bv) is recovered from the v-evacuation's accum_out.
The final out = q*(gamma*s) + x runs on GpSimd (Pool) so the DVE
queue never stalls the next pair's PSUM turnaround.

All matmul operands are bf16 (hosts casts x with round-to-nearest);
PSUM accumulation stays fp32.
"""

import os

import numpy as np
import ml_dtypes

import concourse.bacc as bacc
import concourse.bass as bass
import concourse.mybir as mybir
import concourse.tile as tile
from concourse import bass_utils

F32 = mybir.dt.float32
BF16 = mybir.dt.bfloat16
ALU = mybir.AluOpType
ACT = mybir.ActivationFunctionType

B, C, T, W, H = 8, 64, 16, 64, 64
WP, HP = W + 2, H + 2          # padded slice dims
NPAIR = int(os.environ.get("BASS_NPAIR", T // 2))  # slice pairs per core
RB = 8                         # W-rows per pixel block
NBLK = W // RB                 # pixel blocks per slice
BN = RB * H                    # moving free dim per matmul (512)
NABUF = 4                      # A-tile buffers per parity (4-deep rotation)
FINCH = 4                      # final-pass chunks per slice


def _bf16(a):
    return np.asarray(a, np.float32).astype(ml_dtypes.bfloat16)


def _pack_weights(wq, wk, wv):
    """Pack stationary operands (bf16).

    Moving-data convention: a pair matmul reads the full 128-partition AP
    at (r0=j*RB+dy, dx=0): on even slices the low half (shift copy)
    delivers tap (dy,1) and the high half (plain) tap (dy,0); odd slices
    are mirrored. Single matmuls read the shift copy at dx=1 -> tap
    (dy,2): even from partitions 0-63, odd from 64-127.
    kq column layout: even [Wk | Wq] (q lands on PSUM 64:128 = the
    x-plain half), odd [Wq | Wk].
    """
    def taps(w):  # [O, I, 1, 3, 3] -> tap(dy,dx) = [I, O]
        return np.ascontiguousarray(w.reshape(C, C, 3, 3).transpose(1, 2, 3, 0),
                                    np.float32)

    q_t, k_t, v_t = taps(wq), taps(wk), taps(wv)

    kq_pair = np.zeros((2, 3, 128, 128), np.float32)
    v_pair = np.zeros((2, 3, 128, 64), np.float32)
    for dy in range(3):
        # even parity
        kq_pair[0, dy, 0:64, 0:64] = k_t[:, dy, 1]
        kq_pair[0, dy, 0:64, 64:128] = q_t[:, dy, 1]
        kq_pair[0, dy, 64:128, 0:64] = k_t[:, dy, 0]
        kq_pair[0, dy, 64:128, 64:128] = q_t[:, dy, 0]
        v_pair[0, dy, 0:64, :] = v_t[:, dy, 1]
        v_pair[0, dy, 64:128, :] = v_t[:, dy, 0]
        # odd parity
        kq_pair[1, dy, 0:64, 0:64] = q_t[:, dy, 0]
        kq_pair[1, dy, 0:64, 64:128] = k_t[:, dy, 0]
        kq_pair[1, dy, 64:128, 0:64] = q_t[:, dy, 1]
        kq_pair[1, dy, 64:128, 64:128] = k_t[:, dy, 1]
        v_pair[1, dy, 0:64, :] = v_t[:, dy, 0]
        v_pair[1, dy, 64:128, :] = v_t[:, dy, 1]

    kq_sing = np.zeros((3, 128, 128), np.float32)
    v_sing = np.zeros((3, 128, 64), np.float32)
    for dy in range(3):
        kq_sing[dy, 0:64, 0:64] = k_t[:, dy, 2]
        kq_sing[dy, 0:64, 64:128] = q_t[:, dy, 2]
        kq_sing[dy, 64:128, 0:64] = q_t[:, dy, 2]
        kq_sing[dy, 64:128, 64:128] = k_t[:, dy, 2]
        v_sing[dy, 0:64, :] = v_t[:, dy, 2]
        v_sing[dy, 64:128, :] = v_t[:, dy, 2]

    return _bf16(kq_pair), _bf16(kq_sing), _bf16(v_pair), _bf16(v_sing)


def _emit(nc, tc, x_d, xp_d, wkqp_d, wkqs_d, wvp_d, wvs_d, bias_d, out_d,
          ctx):
    const = ctx.enter_context(tc.tile_pool(name="const", bufs=1))
    state = ctx.enter_context(tc.tile_pool(name="state", bufs=1))
    psum = ctx.enter_context(
        tc.tile_pool(name="psum", bufs=3, space=bass.MemorySpace.PSUM))
    psumv = ctx.enter_context(
        tc.tile_pool(name="psumv", bufs=2, space=bass.MemorySpace.PSUM))
    vpool = ctx.enter_context(tc.tile_pool(name="vpool", bufs=2))

    wkqp_t = const.tile([128, 2, 3, 128], BF16, tag="wkqp")
    wkqs_t = const.tile([128, 3, 128], BF16, tag="wkqs")
    wvp_t = const.tile([128, 2, 3, 64], BF16, tag="wvp")
    wvs_t = const.tile([128, 3, 64], BF16, tag="wvs")
    bias_t = const.tile([128, 4], F32, tag="bias")  # bq, bv, bk*gam, gam

    nc.sync.dma_start(wkqp_t[:], wkqp_d[:])
    nc.sync.dma_start(wkqs_t[:], wkqs_d[:])
    nc.sync.dma_start(wvp_t[:], wvp_d[:])
    nc.sync.dma_start(wvs_t[:], wvs_d[:])
    nc.sync.dma_start(bias_t[:], bias_d[:])

    # A tiles: [shift | plain] for even slices, [plain | shift] for odd.
    ae = [state.tile([128, WP, HP], BF16, tag=f"ae{i}", name=f"ae{i}")
          for i in range(NABUF)]
    ao = [state.tile([128, WP, HP], BF16, tag=f"ao{i}", name=f"ao{i}")
          for i in range(NABUF)]
    qs = [state.tile([128, W * H], BF16, tag=f"qs{i}", name=f"qs{i}")
          for i in range(2)]
    ot = [state.tile([128, W * H], BF16, tag=f"ot{i}", name=f"ot{i}")
          for i in range(2)]
    scr = state.tile([128, BN], F32, tag="scr")
    sparts = [state.tile([128, NBLK], F32, tag=f"sp{i}", name=f"sp{i}")
              for i in range(2)]
    vsum = [state.tile([128, NBLK], F32, tag=f"vs{i}", name=f"vs{i}")
            for i in range(2)]
    t1 = [state.tile([128, 1], F32, tag=f"t1{i}", name=f"t1{i}")
          for i in range(2)]
    vs1 = [state.tile([128, 1], F32, tag=f"vs1{i}", name=f"vs1{i}")
           for i in range(2)]
    sg = [state.tile([128, 1], F32, tag=f"sg{i}", name=f"sg{i}")
          for i in range(2)]
    sf = [state.tile([128, 1], F32, tag=f"sf{i}", name=f"sf{i}")
          for i in range(2)]

    # plain-x tile for the full-width final pass: odd slice on partitions
    # 0-63, even slice on 64-127 (matches q/s/out halves)
    xpl = [state.tile([128, W * H], BF16, tag=f"xpl{i}", name=f"xpl{i}")
           for i in range(NABUF)]

    def load_pair(p):
        # host pre-padded slices: one contiguous descriptor per partition.
        # gpsimd/scalar queues issue async DMA triggers (~600ns); the sync
        # queue blocks for the whole transfer, so no bulk loads there.
        nc.gpsimd.dma_start(ae[p % NABUF][:], xp_d[2 * p])
        nc.gpsimd.dma_start(ao[p % NABUF][:], xp_d[2 * p + 1])

    def load_xpl(p):
        nc.scalar.dma_start(xpl[p % NABUF][0:64, :], x_d[:, 2 * p + 1])
        nc.scalar.dma_start(xpl[p % NABUF][64:128, :], x_d[:, 2 * p])

    load_pair(0)
    if NPAIR > 1:
        load_pair(1)
    load_xpl(0)
    if NPAIR > 1:
        load_xpl(1)

    kvo_mode = os.environ.get("BASS_KVO", "stt")
    fin_eng = nc.vector  # Pool lacks TensorScalarPtr; bf16 gives 2x DVE rate

    def emit_final(p, nch=FINCH):
        # out = q*(gamma*s) + x, full 128-partition DVE ops (both slices
        # at once; xpl interleaves odd/even plain x), store per chunk
        pb = p % 2
        cw = W // nch
        cn = cw * H
        for c_ in range(nch):
            fin_eng.scalar_tensor_tensor(
                out=ot[pb][:, c_ * cn:(c_ + 1) * cn],
                in0=qs[pb][:, c_ * cn:(c_ + 1) * cn],
                scalar=sf[pb][:, 0:1],
                in1=xpl[p % NABUF][:, c_ * cn:(c_ + 1) * cn],
                op0=ALU.mult, op1=ALU.add)
            nc.gpsimd.dma_start(out_d[:, 2 * p, c_ * cw:(c_ + 1) * cw, :],
                                ot[pb][64:128, c_ * cn:(c_ + 1) * cn])
            nc.sync.dma_start(out_d[:, 2 * p + 1, c_ * cw:(c_ + 1) * cw, :],
                              ot[pb][0:64, c_ * cn:(c_ + 1) * cn])

    for p in range(NPAIR):
        pb = p % 2
        ae_, ao_ = ae[p % NABUF], ao[p % NABUF]
        qs_, ot_ = qs[pb], ot[pb]

        for j in range(NBLK):
            kqE = psum.tile([128, BN], F32, tag="kqE", name="kqE")
            kqO = psum.tile([128, BN], F32, tag="kqO", name="kqO")
            V = psumv.tile([128, BN], F32, tag="V", name="V")

            def mov(tl, dy, dx, lo=None):
                r0 = j * RB + dy
                if lo is None:
                    return tl[:, r0:r0 + RB, dx:dx + H]
                if lo:
                    return tl[0:64, r0:r0 + RB, dx:dx + H]
                return tl[64:128, r0:r0 + RB, dx:dx + H]

            # kq pair taps: K=128, full array, serial
            for dy in range(3):
                nc.tensor.matmul(kqE[:, :], wkqp_t[:, 0, dy, :],
                                 mov(ae_, dy, 0), start=(dy == 0), stop=False)
                nc.tensor.matmul(kqO[:, :], wkqp_t[:, 1, dy, :],
                                 mov(ao_, dy, 0), start=(dy == 0), stop=False)
            # kq single taps: K=64, even rows 0-63 / odd rows 64-127,
            # 2-way row-tiled concurrent
            for dy in range(3):
                nc.tensor.matmul(kqE[:, :], wkqs_t[0:64, dy, :],
                                 mov(ae_, dy, 1, lo=True),
                                 start=False, stop=(dy == 2))
                nc.tensor.matmul(kqO[:, :], wkqs_t[64:128, dy, :],
                                 mov(ao_, dy, 1, lo=False),
                                 start=False, stop=(dy == 2))
            # v pair taps: K=128 M=64, even cols 0-63 / odd cols 64-127,
            # 2-way col-tiled concurrent
            for dy in range(3):
                nc.tensor.matmul(V[0:64, :], wvp_t[:, 0, dy, :],
                                 mov(ae_, dy, 0), start=(dy == 0), stop=False,
                                 skip_group_check=True)
                nc.tensor.matmul(V[64:128, :], wvp_t[:, 1, dy, :],
                                 mov(ao_, dy, 0), start=(dy == 0), stop=False,
                                 skip_group_check=True)
            # v single taps: K=64 M=64, quadrants (0,0) and (64,64)
            for dy in range(3):
                nc.tensor.matmul(V[0:64, :], wvs_t[0:64, dy, :],
                                 mov(ae_, dy, 1, lo=True),
                                 start=False, stop=(dy == 2),
                                 skip_group_check=True)
                nc.tensor.matmul(V[64:128, :], wvs_t[64:128, dy, :],
                                 mov(ao_, dy, 1, lo=False),
                                 start=False, stop=(dy == 2),
                                 skip_group_check=True)

            # PSUM evacuation on ScalarE with bias folding; v first — the
            # kv STTs (critical path to freeing kq psum) consume it
            vsb = vpool.tile([128, BN], F32, tag="vsb", name="vsb")
            nc.scalar.activation(vsb[:, :], V[:, :], ACT.Identity,
                                 bias=bias_t[:, 1:2],
                                 accum_out=vsum[pb][:, j:j + 1])
            nc.scalar.activation(qs_[64:128, j * BN:(j + 1) * BN],
                                 kqE[64:128, :], ACT.Identity,
                                 bias=bias_t[64:128, 0:1])
            nc.scalar.activation(qs_[0:64, j * BN:(j + 1) * BN],
                                 kqO[0:64, :], ACT.Identity,
                                 bias=bias_t[0:64, 0:1])

            # fused (gamma*k)*v' multiply + pixel-sum on DVE
            nc.vector.scalar_tensor_tensor(
                out=scr[0:64, :], in0=kqE[0:64, :], scalar=bias_t[0:64, 3:4],
                in1=vsb[0:64, :], op0=ALU.mult, op1=ALU.mult,
                accum_out=sparts[pb][0:64, j:j + 1])
            if kvo_mode == "stt":
                nc.vector.scalar_tensor_tensor(
                    out=scr[64:128, :], in0=kqO[64:128, :],
                    scalar=bias_t[64:128, 3:4],
                    in1=vsb[64:128, :], op0=ALU.mult, op1=ALU.mult,
                    accum_out=sparts[pb][64:128, j:j + 1])
            else:
                nc.vector.scalar_tensor_tensor(
                    out=scr[64:128, :], in0=kqO[64:128, :],
                    scalar=bias_t[64:128, 3:4],
                    in1=vsb[64:128, :], op0=ALU.mult, op1=ALU.mult)
                nc.vector.reduce_sum(sparts[pb][64:128, j:j + 1],
                                     scr[64:128, :],
                                     axis=mybir.AxisListType.X)

            # previous pair's final pass, deferred so the DVE queue never
            # holds this pair's psum turnaround behind it; prefetch loads
            # likewise emitted mid-pair
            if j == 1 and p > 0:
                emit_final(p - 1)
            if j == 3 and p + 2 < NPAIR:
                load_pair(p + 2)
            if j == 5 and p + 2 < NPAIR:
                load_xpl(p + 2)

        # s = gamma*sum(k*v') + gamma*bk*sum(v'), then swap halves so the
        # even-slice s reaches partitions 64-127 (q/x-plain live there)
        nc.vector.reduce_sum(t1[pb][:, :], sparts[pb][:, :],
                             axis=mybir.AxisListType.X)
        nc.vector.reduce_sum(vs1[pb][:, :], vsum[pb][:, :],
                             axis=mybir.AxisListType.X)
        nc.vector.scalar_tensor_tensor(
            out=sg[pb][:, :], in0=vs1[pb][:, :], scalar=bias_t[:, 2:3],
            in1=t1[pb][:, :], op0=ALU.mult, op1=ALU.add)
        nc.scalar.dma_start(sf[pb][64:128, :], sg[pb][0:64, :])
        nc.scalar.dma_start(sf[pb][0:64, :], sg[pb][64:128, :])

    emit_final(NPAIR - 1)


_CACHE = {}


def _build():
    if "nc" in _CACHE:
        return _CACHE["nc"]
    nc = bacc.Bacc("TRN2", target_bir_lowering=False, debug=False,
                   enable_asserts=False, num_devices=8)
    x_d = nc.dram_tensor("x", (C, T, W, H), BF16, kind="ExternalInput").ap()
    xp_d = nc.dram_tensor("xp", (T, 128, WP, HP), BF16,
                          kind="ExternalInput").ap()
    wkqp_d = nc.dram_tensor("wkqp", (128, 2, 3, 128), BF16,
                            kind="ExternalInput").ap()
    wkqs_d = nc.dram_tensor("wkqs", (128, 3, 128), BF16,
                            kind="ExternalInput").ap()
    wvp_d = nc.dram_tensor("wvp", (128, 2, 3, 64), BF16,
                           kind="ExternalInput").ap()
    wvs_d = nc.dram_tensor("wvs", (128, 3, 64), BF16,
                           kind="ExternalInput").ap()
    bias_d = nc.dram_tensor("bias", (128, 4), F32, kind="ExternalInput").ap()
    out_d = nc.dram_tensor("out", (C, T, W, H), BF16,
                           kind="ExternalOutput").ap()
    from contextlib import ExitStack
    with tile.TileContext(nc) as tc, ExitStack() as ctx:
        _emit(nc, tc, x_d, xp_d, wkqp_d, wkqs_d, wvp_d, wvs_d, bias_d, out_d,
              ctx)
    nc.compile()
    _CACHE["nc"] = nc
    return nc


def run_spmd(x, wq, wk, wv, bq, bk, bv, gamma, trace=False, **kw):
    nc = _build()
    wkqp, wkqs, wvp, wvs = _pack_weights(
        np.asarray(wq, np.float32), np.asarray(wk, np.float32),
        np.asarray(wv, np.float32))
    # stationary tiles are [128(K), ...free...]: transpose packed
    # [..., K, M] so K is the partition dim
    wkqp = np.ascontiguousarray(wkqp.transpose(2, 0, 1, 3))   # [128,2,3,128]
    wkqs = np.ascontiguousarray(wkqs.transpose(1, 0, 2))      # [128,3,128]
    wvp = np.ascontiguousarray(wvp.transpose(2, 0, 1, 3))     # [128,2,3,64]
    wvs = np.ascontiguousarray(wvs.transpose(1, 0, 2))        # [128,3,64]

    gam = np.float32(np.asarray(gamma).reshape(-1)[0])
    bias = np.zeros((128, 4), np.float32)
    bias[0:64, 0] = bias[64:128, 0] = np.asarray(bq, np.float32)
    bias[0:64, 1] = bias[64:128, 1] = np.asarray(bv, np.float32)
    bias[0:64, 2] = bias[64:128, 2] = np.asarray(bk, np.float32) * gam
    bias[:, 3] = gam

    xb = _bf16(x)
    # host pre-padded per-slice layout [T, 128, WP, HP]: even slices
    # [shift | plain], odd slices [plain | shift] on the partition halves
    zpad = np.zeros((B, T, C, WP, HP), ml_dtypes.bfloat16)
    zsh = np.zeros((B, T, C, WP, HP), ml_dtypes.bfloat16)
    xt = xb.transpose(0, 2, 1, 3, 4)            # [B, T, C, W, H]
    zpad[:, :, :, 1:1 + W, 1:1 + H] = xt
    zsh[:, :, :, 1:1 + W, 0:H] = xt
    xp = np.empty((B, T, 128, WP, HP), ml_dtypes.bfloat16)
    xp[:, 0::2, 0:64] = zsh[:, 0::2]
    xp[:, 0::2, 64:128] = zpad[:, 0::2]
    xp[:, 1::2, 0:64] = zpad[:, 1::2]
    xp[:, 1::2, 64:128] = zsh[:, 1::2]
    in_maps = [
        {"x": np.ascontiguousarray(xb[b]), "xp": np.ascontiguousarray(xp[b]),
         "wkqp": wkqp, "wkqs": wkqs,
         "wvp": wvp, "wvs": wvs, "bias": bias}
        for b in range(B)
    ]
    res = bass_utils.run_bass_kernel_spmd(
        nc, in_maps, core_ids=list(range(B)), trace=trace, **kw)
    out = np.stack([np.asarray(res.results[b]["out"]).astype(np.float32)
                    for b in range(B)], axis=0)
    return out, res


def kernel(x, wq, wk, wv, bq, bk, bv, gamma):
    out, _ = run_spmd(x, wq, wk, wv, bq, bk, bv, gamma)
    return out
